# revision 34
# baseline (speedup 1.0000x reference)
"""Causal self-attention Trainium2 kernel, v6.

Full inputs in, full output out. 8 NeuronCores: data-parallel on batch (2) x
tensor-parallel on heads (4 groups of 4 heads = 2 pairs). Transposed layout
(head-dim / key-dim on partitions).

Key facts learned on hw: matmul time = out-columns x 1 cycle regardless of
dtype; fp8 DoubleRow only halves matmuls that ACCUMULATE over contraction
tiles (kqv: 8 dc steps -> 4; proj 2 -> 1). S is single-shot -> bf16. PV
would need v+P in fp8 (fails the 2e-2 accuracy gate) -> bf16.

vs v3:
- k/q projections via fp8e4m3 DoubleRow (x and Wk/Wq fp8, ALPHA=32
  pre-scale): 4 accumulation steps instead of 8. Output kqvT stays bf16;
  S/PV/exp numerics bf16 (err ~1.2% << 2e-2).
- v computed directly in [m, dh] layout (lhsT = x^T block, rhs = Wv bf16):
  no PE transposes, no v-kqv section; v bias folded into b_proj on host.
- masks shrunk to the 128-wide diagonal triangle, applied in-place on P.
- exp scale = 0.125/ALPHA^2 compensates the k/q pre-scale.

Per-core device program (fp32 PSUM):
  k^T/q^T = Wf.T @dr xf (+bias on DVE fp32->bf16 copy-out)  [fp8 DoubleRow]
  vp = x^T-block.T @ Wv  [m, 4x(64 dh|1)]                   [bf16]
  per head: S^T = k^T-block.T @ q^T-chunk  [128m x 512n]    [bf16]
            P^T = exp(0.125/a^2 * S^T); in-place tri-mask on diag
            U^T = [v|1].T-block @ P^T    rows 0-63 sa^T raw, row 64 denom
            sa^T = U^T[0:64] * recip(bcast denom)
  partial out^T = WprojT.T @ sa^T [1024, 2048] bf16 -> DRAM
"""
import sys, os
sys.path.insert(0, '/opt/trn_rl_repo')
os.environ.setdefault("JAX_PLATFORMS", "")

import numpy as np
import ml_dtypes

import concourse.bass as bass
import concourse.bacc as bacc
import concourse.tile as tile
import concourse.mybir as mybir
from concourse import bass_utils

B, N, D, H, DH = 2, 2048, 1024, 16, 64
G = 4              # heads per core
NCORES = 8
NCH = 512          # n-chunk width
NJ = N // NCH      # 4 n-chunks
NMB = N // 128     # 16 m-blocks
ALPHA = 32.0       # host k/q pre-scale (fp8 subnormal dodge)
bf16 = ml_dtypes.bfloat16
f8 = ml_dtypes.float8_e4m3
f32 = np.float32
AF = mybir.ActivationFunctionType
DR = mybir.MatmulPerfMode.DoubleRow

_cache = {}


def _build_program():
    nc = bacc.Bacc("TRN2", target_bir_lowering=False, debug=False, num_devices=NCORES)
    dt = mybir.dt

    xf_d = nc.dram_tensor("xf", [4, 128, 2, N], dt.float8e4, kind="ExternalInput").ap()
    xt_d = nc.dram_tensor("xt", [D, N], dt.bfloat16, kind="ExternalInput").ap()
    # 4 k/q groups: (pair, sec): [4dc', 128, 2kt, 128] fp8
    w_d = nc.dram_tensor("w", [4, 4, 128, 2, 128], dt.float8e4, kind="ExternalInput").ap()
    b_d = nc.dram_tensor("bvec", [4, 128], dt.float32, kind="ExternalInput").ap()
    wv_d = nc.dram_tensor("wv", [8, 128, 256], dt.bfloat16, kind="ExternalInput").ap()
    wpt_d = nc.dram_tensor("wpt", [2 * 128, D], dt.bfloat16, kind="ExternalInput").ap()
    mask_d = nc.dram_tensor("masks", [128, 256], dt.bfloat16, kind="ExternalInput").ap()
    ones_d = nc.dram_tensor("ones", [128, 64], dt.float32r, kind="ExternalInput").ap()
    out_d = nc.dram_tensor("outt", [D, N], dt.bfloat16, kind="ExternalOutput").ap()
    dbg = None
    if os.environ.get("KDBG") == "1":
        dbg = {
            "sa": nc.dram_tensor("dbg_sa", [2, 128, N], dt.bfloat16, kind="ExternalOutput").ap(),
            "kqvT": nc.dram_tensor("dbg_kqvT", [4, 128, N], dt.bfloat16, kind="ExternalOutput").ap(),
            "vp": nc.dram_tensor("dbg_vp", [128, 16 * 264], dt.bfloat16, kind="ExternalOutput").ap(),
        }

    with tile.TileContext(nc) as tc:
        _emit(nc, tc, xf_d, xt_d, w_d, b_d, wv_d, wpt_d, mask_d, ones_d, out_d, dbg)

    nc.compile()
    return nc


def _emit(nc, tc, xf_d, xt_d, w_d, b_d, wv_d, wpt_d, mask_d, ones_d, out_d, dbg=None):
    from contextlib import ExitStack

    dt = mybir.dt
    ctx = ExitStack()
    with ctx:
        consts = ctx.enter_context(tc.tile_pool(name="consts", bufs=1))
        work = ctx.enter_context(tc.tile_pool(name="work", bufs=1))

        # ---- staged constant loads, consumption order ----
        w_sb = [None] * 4

        def load_w(mc):
            w_sb[mc] = consts.tile([128, 4, 2, 128], dt.float8e4, name=f"w{mc}", tag=f"w{mc}")
            nc.sync.dma_start(w_sb[mc][:], w_d[mc].transpose([1, 0, 2, 3]))

        # fp8 x^T folded, for k/q DR matmuls: quarters then half
        xf_q = [[None] * 2 for _ in range(4)]
        xf_half = [None] * 4

        def load_xf_quarter(sub):
            for dc in range(4):
                t = consts.tile([128, 2, NCH], dt.float8e4,
                                name=f"xfq{dc}_{sub}", tag=f"xfq{dc}_{sub}")
                nc.sync.dma_start(t[:], xf_d[dc][:, :, sub * NCH:(sub + 1) * NCH])
                xf_q[dc][sub] = t

        def load_xf_half():
            for dc in range(4):
                t = consts.tile([128, 2, N // 2], dt.float8e4,
                                name=f"xfh{dc}", tag=f"xfh{dc}")
                nc.sync.dma_start(t[:], xf_d[dc][:, :, N // 2:])
                xf_half[dc] = t

        def xf_ap(dc, jj):
            if jj < 2:
                return xf_q[dc][jj][:]
            return xf_half[dc][:, :, (jj - 2) * NCH:(jj - 1) * NCH]

        # bf16 x^T, for v-direct: quarters then half
        xt_q = [[None] * 2 for _ in range(8)]
        xt_sb = [None] * 8

        def load_xt_quarter(sub):
            for dc in range(8):
                t = consts.tile([128, NCH], dt.bfloat16,
                                name=f"xtq{dc}_{sub}", tag=f"xtq{dc}_{sub}")
                nc.sync.dma_start(
                    t[:], xt_d[dc * 128:(dc + 1) * 128,
                               sub * NCH:(sub + 1) * NCH])
                xt_q[dc][sub] = t

        def load_xt_half():
            for dc in range(8):
                t = consts.tile([128, N // 2], dt.bfloat16,
                                name=f"xth{dc}", tag=f"xth{dc}")
                nc.sync.dma_start(
                    t[:], xt_d[dc * 128:(dc + 1) * 128, N // 2:])
                xt_sb[dc] = t

        def xt_ap(dc, jj):
            if jj < 2:
                return xt_q[dc][jj][:]
            return xt_sb[dc][:, (jj - 2) * NCH:(jj - 1) * NCH]

        load_w(0)
        load_w(1)
        load_xf_quarter(0)
        b_sb = consts.tile([128, 4], dt.float32, name="ball", tag="ball")
        nc.sync.dma_start(b_sb[:], b_d.transpose([1, 0]))
        wv_sb = consts.tile([128, 8, 256], dt.bfloat16, name="wv", tag="wv")
        nc.sync.dma_start(wv_sb[:], wv_d.transpose([1, 0, 2]))
        load_xt_quarter(0)
        load_w(2)
        load_w(3)
        mask_sb = consts.tile([128, 256], dt.bfloat16, name="maskall", tag="maskall")
        nc.sync.dma_start(mask_sb[:], mask_d[:])
        ones_sb = consts.tile([128, 64], dt.float32r, name="ones", tag="ones")
        nc.sync.dma_start(ones_sb[:], ones_d[:])
        load_xf_quarter(1)
        load_xt_quarter(1)
        load_xf_half()
        load_xt_half()
        wpt_sb = []
        for kc in range(2):
            t = consts.tile([128, D], dt.bfloat16, name=f"wpt{kc}", tag=f"wpt{kc}")
            nc.sync.dma_start(t[:], wpt_d[kc * 128:(kc + 1) * 128, :])
            wpt_sb.append(t)

        # ---- persistent work tiles ----
        # kqvT[2p+sec]: sec 0=k, 1=q for pair p; [he-dh64 | ho-dh64] rows
        kqvT = [work.tile([128, N], dt.bfloat16, name=f"kqvT{i}", tag=f"kqvT{i}")
                for i in range(4)]
        # vp: [m-part, mb, 4*(64 dh + ones + pad)] bf16
        vp = work.tile([128, NMB, 4 * 66], dt.bfloat16, name="vp", tag="vp")
        saT = [work.tile([128, N], dt.bfloat16, name=f"saT{kc}", tag=f"saT{kc}")
               for kc in range(2)]
        for h in range(G):
            nc.gpsimd.memset(vp[:, :, 66 * h + 64:66 * h + 65], 1.0)

        # ---- pools ----
        ps = ctx.enter_context(tc.tile_pool(name="ps", bufs=2, space="PSUM"))
        pu = ctx.enter_context(tc.tile_pool(name="pu", bufs=2, space="PSUM"))
        pp = ctx.enter_context(tc.tile_pool(name="pp", bufs=2, space="PSUM"))
        pPool = ctx.enter_context(tc.tile_pool(name="pP", bufs=9))
        pun = ctx.enter_context(tc.tile_pool(name="pun", bufs=5))
        paux = ctx.enter_context(tc.tile_pool(name="paux", bufs=6))
        pout = ctx.enter_context(tc.tile_pool(name="pout", bufs=6))

        def emit_kq_group(mc, jj):
            """mc = 2*pair + sec (sec 0=k, 1=q). fp8 DR, bf16 copy-out+bias."""
            ps_t = pp.tile([128, NCH], dt.float32, tag="pp", name="kqp")
            for dc in range(4):
                nc.tensor.matmul(
                    ps_t[:],
                    w_sb[mc][:, dc, :, :],
                    xf_ap(dc, jj),
                    start=(dc == 0), stop=(dc == 3),
                    perf_mode=DR,
                )
            nc.vector.tensor_scalar_add(
                kqvT[mc][:, jj * NCH:(jj + 1) * NCH], ps_t[:], b_sb[:, mc:mc + 1])

        def emit_vd(mb):
            """v-direct: vp[:, mb, 4x64] = x^T-block.T @ Wv."""
            ps_t = pp.tile([128, 256], dt.float32, tag="pp", name="vdp")
            jj = mb // 4
            for dc in range(8):
                nc.tensor.matmul(
                    ps_t[:],
                    xt_ap(dc, jj)[:, (mb % 4) * 128:(mb % 4 + 1) * 128],
                    wv_sb[:, dc, :],
                    start=(dc == 0), stop=(dc == 7),
                )
            nc.vector.tensor_copy(
                vp[:, mb, :].rearrange("p (h e) -> p h e", h=4)[:, :, 0:64],
                ps_t[:].rearrange("p (h e) -> p h e", h=4))

        def head_slices(h):
            p, o = h // 2, (h % 2) * 64
            kT = kqvT[2 * p][o:o + 64, :]
            qT = kqvT[2 * p + 1][o:o + 64, :]
            return kT, qT, o

        def emit_proj_oc(j, oc, scalar_copy=False):
            nsl = slice(j * NCH, (j + 1) * NCH)
            pp_t = pp.tile([128, NCH], dt.float32, tag="pp", name="pp_t")
            for kc in range(2):
                nc.tensor.matmul(
                    pp_t[:],
                    wpt_sb[kc][:, oc * 128:(oc + 1) * 128],
                    saT[kc][:, nsl],
                    start=(kc == 0), stop=(kc == 1),
                )
            o_t = pout.tile([128, NCH], dt.bfloat16, tag="o", name="o_t")
            if scalar_copy:
                nc.scalar.copy(o_t[:], pp_t[:])
            else:
                nc.vector.tensor_copy(o_t[:], pp_t[:])
            nc.sync.dma_start(out_d[oc * 128:(oc + 1) * 128, nsl], o_t[:])

        def norm_steps(h, j, u_t, dve_mul=False):
            kc, row = h // 2, (h % 2) * 64
            nsl = slice(j * NCH, (j + 1) * NCH)
            st = {}
            mul_eng = nc.vector if dve_mul else nc.gpsimd

            def s1():
                st["u_sb"] = pun.tile([65, NCH], dt.float32r, tag="un", name="usb")
                nc.vector.tensor_copy(st["u_sb"][0:65, :], u_t[0:65, :])

            def s2():
                bt = ps.tile([128, 2 * NCH], dt.float32, tag="s2", name="bcp")
                st["bcp"] = bt[0:64, 0:NCH]
                nc.tensor.matmul(
                    st["bcp"],
                    ones_sb[64:65, 0:64],
                    st["u_sb"][64:65, :],
                    start=True, stop=True,
                )

            def s3():
                st["rc"] = paux.tile([64, NCH], dt.float32, tag="rc", name="rc")
                nc.vector.reciprocal_approx_fast(st["rc"][:], st["bcp"])

            def s4():
                u_f32 = st["u_sb"][0:64, :].bitcast(dt.float32)
                if row == 0:
                    mul_eng.tensor_mul(saT[kc][0:64, nsl], u_f32, st["rc"][:])
                else:
                    tmp = paux.tile([64, NCH], dt.bfloat16, tag="tmp", name="tmp")
                    mul_eng.tensor_mul(tmp[:], u_f32, st["rc"][:])
                    nc.sync.dma_start(saT[kc][64:128, nsl], tmp[:])

            return [s1, s2, s3, s4]

        def emit_attn_chunk(j, p, fillers, norm_prev, gated=(), norm_dve=False):
            """Attention chunk j for pair p. `fillers`: dependency-free PE work.
            `norm_prev`: deferred norm steps of the previous chunk. `gated`:
            fillers that must be EMITTED only after all norm_prev steps (they
            read tiles norm_prev writes)."""
            nm = 4 * (j + 1)
            pair = (2 * p, 2 * p + 1)
            u_t = {h: pu.tile([65, NCH], dt.float32, tag="u", name=f"u{h}")
                   for h in pair}
            p_tiles = {h: [None] * nm for h in pair}
            offs = [0] * nm
            from itertools import zip_longest
            units = [u for pair_ in zip_longest(list(norm_prev), list(fillers))
                     for u in pair_ if u is not None]
            units += list(gated)
            total = len(units)
            acc = [0]

            def drain_share():
                acc[0] += total
                while acc[0] >= nm and units:
                    acc[0] -= nm
                    units.pop(0)()

            def pair_view(t, off):
                return t[:, :].rearrange("p (i n) -> p i n", i=2)[:, :, off:]

            def emit_s_pair(mi):
                r = mi - 4 * j
                off = 128 * r if r > 0 else 0
                offs[mi] = off
                s2t = ps.tile([128, 2 * NCH], dt.float32, tag="s2", name="s2t")
                for idx, h in enumerate(pair):
                    kT, qT, _ = head_slices(h)
                    nc.tensor.matmul(
                        s2t[:, idx * NCH + off:(idx + 1) * NCH],
                        kT[:, mi * 128:(mi + 1) * 128],
                        qT[:, j * NCH + off:(j + 1) * NCH],
                        start=True, stop=True, skip_group_check=True,
                    )
                p_t = pPool.tile([128, 2 * NCH], dt.bfloat16, tag="p", name="p_t")
                nc.scalar.activation(pair_view(p_t, off), pair_view(s2t, off),
                                     AF.Exp, scale=0.125 / (ALPHA * ALPHA))
                if r >= 0:
                    # in-place triangular mask on the 128-wide diagonal band
                    band = pair_view(p_t, off)[:, :, 0:128]
                    nc.vector.tensor_mul(
                        band, band,
                        mask_sb[:, :].rearrange("p (i n) -> p i n", i=2))
                for h in pair:
                    p_tiles[h][mi] = p_t

            def emit_pv(h, mi):
                off = offs[mi]
                idx = h % 2
                nc.tensor.matmul(
                    u_t[h][:, off:],
                    vp[:, mi, 66 * h:66 * h + 65],
                    p_tiles[h][mi][:, idx * NCH + off:(idx + 1) * NCH],
                    start=(mi == 0), stop=(mi == nm - 1),
                    skip_group_check=True,
                )

            depth = 4
            for mi in range(nm):
                drain_share()
                emit_s_pair(mi)
                if mi >= depth:
                    for h in pair:
                        emit_pv(h, mi - depth)
            for mi in range(max(nm - depth, 0), nm):
                for h in pair:
                    emit_pv(h, mi)
            while units:
                units.pop(0)()

            return [st for h in pair
                    for st in norm_steps(h, j, u_t[h], dve_mul=norm_dve)]

        HCH = NCH // 2  # 256

        def emit_proj_half(h2, oc, scalar_copy=False):
            nsl = slice(h2 * HCH, (h2 + 1) * HCH)
            pp_t = pp.tile([128, NCH], dt.float32, tag="pp", name="pp_t")
            for kc in range(2):
                nc.tensor.matmul(
                    pp_t[:, 0:HCH],
                    wpt_sb[kc][:, oc * 128:(oc + 1) * 128],
                    saT[kc][:, nsl],
                    start=(kc == 0), stop=(kc == 1),
                )
            o_t = pout.tile([128, HCH], dt.bfloat16, tag="o", name="o_t")
            if scalar_copy:
                nc.scalar.copy(o_t[:], pp_t[:, 0:HCH])
            else:
                nc.vector.tensor_copy(o_t[:], pp_t[:, 0:HCH])
            nc.sync.dma_start(out_d[oc * 128:(oc + 1) * 128, nsl], o_t[:])

        def norm_steps_half(h, h2, u_t):
            kc, row = h // 2, (h % 2) * 64
            nsl = slice(h2 * HCH, (h2 + 1) * HCH)
            st = {}

            def s1():
                st["u_sb"] = pun.tile([65, HCH], dt.float32r, tag="un", name="usb")
                nc.vector.tensor_copy(st["u_sb"][0:65, :], u_t[0:65, :])

            def s2():
                bt = ps.tile([128, 2 * NCH], dt.float32, tag="s2", name="bcp")
                st["bcp"] = bt[0:64, 0:HCH]
                nc.tensor.matmul(
                    st["bcp"],
                    ones_sb[64:65, 0:64],
                    st["u_sb"][64:65, :],
                    start=True, stop=True,
                )

            def s3():
                st["rc"] = paux.tile([64, HCH], dt.float32, tag="rc", name="rc")
                nc.vector.reciprocal_approx_fast(st["rc"][:], st["bcp"])

            def s4():
                u_f32 = st["u_sb"][0:64, :].bitcast(dt.float32)
                if row == 0:
                    nc.vector.tensor_mul(saT[kc][0:64, nsl], u_f32, st["rc"][:])
                else:
                    tmp = paux.tile([64, HCH], dt.bfloat16, tag="tmp", name="tmp")
                    nc.vector.tensor_mul(tmp[:], u_f32, st["rc"][:])
                    nc.sync.dma_start(saT[kc][64:128, nsl], tmp[:])

            return [s1, s2, s3, s4]

        def emit_attn_half(p, h2, fillers, norm_prev, gated=()):
            """256-col half h2 of chunk 0 for pair p (tail shortening).
            Head idx keeps its own NCH-aligned PSUM region (one accumulation
            group per bank, like the full-size path)."""
            nm = 2 * (h2 + 1)
            pair = (2 * p, 2 * p + 1)
            u_t = {h: pu.tile([65, HCH], dt.float32, tag="u", name=f"uh{h}")
                   for h in pair}
            p_tiles = {h: [None] * nm for h in pair}
            offs = [0] * nm
            from itertools import zip_longest
            units = [u for pair_ in zip_longest(list(norm_prev), list(fillers))
                     for u in pair_ if u is not None]
            units += list(gated)
            total = len(units)
            acc = [0]

            def drain_share():
                acc[0] += total
                while acc[0] >= nm and units:
                    acc[0] -= nm
                    units.pop(0)()

            def half_view(t, off):
                return t[:, :].rearrange("p (i n) -> p i n", i=2)[:, :, off:HCH]

            def emit_s_half(mi):
                r = mi - 2 * h2
                off = 128 * r if r > 0 else 0
                offs[mi] = off
                s2t = ps.tile([128, 2 * NCH], dt.float32, tag="s2", name="s2t")
                for idx, h in enumerate(pair):
                    kT, qT, _ = head_slices(h)
                    nc.tensor.matmul(
                        s2t[:, idx * NCH + off:idx * NCH + HCH],
                        kT[:, mi * 128:(mi + 1) * 128],
                        qT[:, h2 * HCH + off:(h2 + 1) * HCH],
                        start=True, stop=True, skip_group_check=True,
                    )
                p_t = pPool.tile([128, 2 * NCH], dt.bfloat16, tag="p", name="p_t")
                nc.scalar.activation(half_view(p_t, off), half_view(s2t, off),
                                     AF.Exp, scale=0.125 / (ALPHA * ALPHA))
                if r >= 0:
                    band = half_view(p_t, off)[:, :, 0:128]
                    nc.vector.tensor_mul(
                        band, band,
                        mask_sb[:, :].rearrange("p (i n) -> p i n", i=2))
                for h in pair:
                    p_tiles[h][mi] = p_t

            def emit_pv(h, mi):
                off = offs[mi]
                idx = h % 2
                nc.tensor.matmul(
                    u_t[h][:, off:],
                    vp[:, mi, 66 * h:66 * h + 65],
                    p_tiles[h][mi][:, idx * NCH + off:idx * NCH + HCH],
                    start=(mi == 0), stop=(mi == nm - 1),
                    skip_group_check=True,
                )

            for mi in range(nm):
                drain_share()
                emit_s_half(mi)
            for mi in range(nm):
                for h in pair:
                    emit_pv(h, mi)
            while units:
                units.pop(0)()

            return [st for h in pair for st in norm_steps_half(h, h2, u_t[h])]

        # ---- window schedule ----
        KQ = emit_kq_group
        VD = emit_vd

        def F(fn, *a):
            return lambda: fn(*a)

        # A-mini: enough for attn (p0, 0): pair0 k/q at jj=0
        KQ(0, 0); KQ(1, 0)

        windows = [
            (0, 0, [F(VD, 0), F(VD, 1), F(VD, 2), F(VD, 3),
                    F(KQ, 2, 0), F(KQ, 3, 0), F(KQ, 0, 1), F(KQ, 1, 1)], []),
            (0, 1, [F(VD, 4), F(VD, 5), F(VD, 6), F(VD, 7),
                    F(KQ, 2, 1), F(KQ, 3, 1), F(KQ, 0, 2), F(KQ, 1, 2)], []),
            (0, 2, [F(VD, 8), F(VD, 9), F(VD, 10), F(VD, 11),
                    F(KQ, 2, 2), F(KQ, 3, 2), F(KQ, 0, 3), F(KQ, 1, 3)], []),
            (0, 3, [F(VD, 12), F(VD, 13), F(VD, 14), F(VD, 15),
                    F(KQ, 2, 3), F(KQ, 3, 3)], []),
            (1, 1, [], []),
            (1, 2, [], [F(emit_proj_oc, 1, oc) for oc in range(4)]),
            (1, 3, [F(emit_proj_oc, 1, oc) for oc in range(4, 8)],
                   [F(emit_proj_oc, 2, oc) for oc in range(6)]),
        ]

        pending_norm = []
        for wi, (p, j, fills, gated) in enumerate(windows):
            pending_norm = emit_attn_chunk(j, p, fills, pending_norm, gated,
                                           norm_dve=False)
        # final chunk (pair1, j=0) split into two 256-col halves: half A's
        # norm + projection hide inside half B; only half B's remain serial
        norm_a = emit_attn_half(
            1, 0, [F(emit_proj_oc, 2, oc) for oc in range(6, 8)], pending_norm,
            [F(emit_proj_oc, 3, oc, oc % 2 == 1) for oc in range(8)])
        norm_b = emit_attn_half(
            1, 1, [], norm_a,
            [F(emit_proj_half, 0, oc, oc % 2 == 1) for oc in range(8)])
        # tail: half B's norm (e/o interleaved) + its projection
        e_st, o_st = norm_b[0:4], norm_b[4:8]
        for a, b in zip(e_st, o_st):
            a(); b()
        for oc in range(8):
            emit_proj_half(1, oc, scalar_copy=(oc % 2 == 1))

        if dbg is not None:
            for kc in range(2):
                nc.sync.dma_start(dbg["sa"][kc], saT[kc][:])
            for i in range(4):
                nc.sync.dma_start(dbg["kqvT"][i], kqvT[i][:])
            nc.sync.dma_start(dbg["vp"], vp.rearrange("p a b -> p (a b)"))


def _host_prep(x, W_kqv, b_kqv, W_proj):
    """Build the 8 per-core input maps."""
    x = np.asarray(x, dtype=f32)
    W_kqv = np.asarray(W_kqv, dtype=f32)
    b_kqv = np.asarray(b_kqv, dtype=f32)
    W_proj = np.asarray(W_proj, dtype=f32)

    # 128-wide triangular diag mask, duplicated for the head pair
    mm = np.arange(128)[:, None]
    cc = np.arange(128)[None, :]
    m1 = (cc >= mm).astype(bf16)
    masks = np.concatenate([m1, m1], axis=1)  # [128, 256]

    in_maps = []
    for c in range(NCORES):
        b, g = c // 4, c % 4
        heads = [4 * g + i for i in range(4)]
        # k/q groups: mc = 2*pair + sec, cols [he-dh64 | ho-dh64], fp8 fold
        wtiles, btiles = [], []
        for p in range(2):
            he, ho = heads[2 * p], heads[2 * p + 1]
            for sec in range(2):  # k, q
                blk = np.concatenate(
                    [W_kqv[h][:, sec * 64:(sec + 1) * 64] for h in (he, ho)],
                    axis=1) * ALPHA  # [1024, 128]
                wtiles.append(blk.reshape(4, 2, 128, 128).transpose(0, 2, 1, 3)
                              .astype(f8))
                btiles.append(np.concatenate(
                    [b_kqv[h][sec * 64:(sec + 1) * 64] for h in (he, ho)]
                ).astype(f32) * ALPHA)
        # Wv: [8, 128, 256], cols = 4 heads x 64 (UNSCALED, bias dropped)
        wv = np.concatenate([W_kqv[h][:, 128:192] for h in heads], axis=1)
        wv = wv.reshape(8, 128, 256).astype(bf16)
        xt = np.ascontiguousarray(x[b].T)  # [1024, 2048]
        xf = xt.reshape(4, 2, 128, N).transpose(0, 2, 1, 3).astype(f8)
        in_maps.append({
            "xf": xf,
            "xt": xt.astype(bf16),
            "w": np.stack(wtiles),
            "bvec": np.stack(btiles),
            "wv": wv,
            "wpt": np.ascontiguousarray(W_proj[:, 256 * g:256 * (g + 1)].T).astype(bf16),
            "masks": masks,
            "ones": np.ones((128, 64), dtype=f32),
        })
    return in_maps


def run(x, W_kqv, b_kqv, W_proj, b_proj, trace=False, trace_cores=None):
    if "nc" not in _cache:
        _cache["nc"] = _build_program()
    nc = _cache["nc"]
    in_maps = _host_prep(x, W_kqv, b_kqv, W_proj)
    res = bass_utils.run_bass_kernel_spmd(
        nc, in_maps, core_ids=list(range(NCORES)),
        trace=trace, trace_cores=trace_cores,
    )
    b_proj = np.asarray(b_proj, dtype=f32)
    W_proj_f = np.asarray(W_proj, dtype=f32)
    # v bias folded: sum_m attn = 1 -> sa += bv; out += bv_all @ W_proj.T
    bv_all = np.asarray(b_kqv, dtype=f32)[:, 128:192].reshape(-1)  # [1024]
    b_eff = b_proj + bv_all @ W_proj_f.T
    out = np.zeros((B, N, D), dtype=f32)
    for b in range(B):
        acc = res.results[4 * b]["outt"].astype(f32)
        for g in range(1, 4):
            acc = acc + res.results[4 * b + g]["outt"].astype(f32)
        out[b] = acc.T + b_eff[None, :]
    return out, res


def kernel(x, W_kqv, b_kqv, W_proj, b_proj):
    out, _ = run(x, W_kqv, b_kqv, W_proj, b_proj, trace=False)
    return out


# revision 35
# speedup vs baseline: 1.0451x; 1.0451x over previous
"""Causal self-attention Trainium2 kernel, v6.

Full inputs in, full output out. 8 NeuronCores: data-parallel on batch (2) x
tensor-parallel on heads (4 groups of 4 heads = 2 pairs). Transposed layout
(head-dim / key-dim on partitions).

Key facts learned on hw: matmul time = out-columns x 1 cycle regardless of
dtype; fp8 DoubleRow only halves matmuls that ACCUMULATE over contraction
tiles (kqv: 8 dc steps -> 4; proj 2 -> 1). S is single-shot -> bf16. PV
would need v+P in fp8 (fails the 2e-2 accuracy gate) -> bf16.

vs v3:
- k/q projections via fp8e4m3 DoubleRow (x and Wk/Wq fp8, ALPHA=32
  pre-scale): 4 accumulation steps instead of 8. Output kqvT stays bf16;
  S/PV/exp numerics bf16 (err ~1.2% << 2e-2).
- v computed directly in [m, dh] layout (lhsT = x^T block, rhs = Wv bf16):
  no PE transposes, no v-kqv section; v bias folded into b_proj on host.
- masks shrunk to the 128-wide diagonal triangle, applied in-place on P.
- exp scale = 0.125/ALPHA^2 compensates the k/q pre-scale.

Per-core device program (fp32 PSUM):
  k^T/q^T = Wf.T @dr xf (+bias on DVE fp32->bf16 copy-out)  [fp8 DoubleRow]
  vp = x^T-block.T @ Wv  [m, 4x(64 dh|1)]                   [bf16]
  per head: S^T = k^T-block.T @ q^T-chunk  [128m x 512n]    [bf16]
            P^T = exp(0.125/a^2 * S^T); in-place tri-mask on diag
            U^T = [v|1].T-block @ P^T    rows 0-63 sa^T raw, row 64 denom
            sa^T = U^T[0:64] * recip(bcast denom)
  partial out^T = WprojT.T @ sa^T [1024, 2048] bf16 -> DRAM
"""
import sys, os
sys.path.insert(0, '/opt/trn_rl_repo')
os.environ.setdefault("JAX_PLATFORMS", "")

import numpy as np
import ml_dtypes

import concourse.bass as bass
import concourse.bacc as bacc
import concourse.tile as tile
import concourse.mybir as mybir
from concourse import bass_utils

B, N, D, H, DH = 2, 2048, 1024, 16, 64
G = 4              # heads per core
NCORES = 8
NCH = 512          # n-chunk width
NJ = N // NCH      # 4 n-chunks
NMB = N // 128     # 16 m-blocks
ALPHA = 32.0       # host k/q pre-scale (fp8 subnormal dodge)
bf16 = ml_dtypes.bfloat16
f8 = ml_dtypes.float8_e4m3
f32 = np.float32
AF = mybir.ActivationFunctionType
DR = mybir.MatmulPerfMode.DoubleRow

_cache = {}


def _build_program():
    nc = bacc.Bacc("TRN2", target_bir_lowering=False, debug=False, num_devices=NCORES)
    dt = mybir.dt

    xf_d = nc.dram_tensor("xf", [4, 128, 2, N], dt.float8e4, kind="ExternalInput").ap()
    xt_d = nc.dram_tensor("xt", [D, N], dt.bfloat16, kind="ExternalInput").ap()
    # 4 k/q groups: (pair, sec): [4dc', 128, 2kt, 128] fp8
    w_d = nc.dram_tensor("w", [4, 4, 128, 2, 128], dt.float8e4, kind="ExternalInput").ap()
    b_d = nc.dram_tensor("bvec", [4, 128], dt.float32, kind="ExternalInput").ap()
    wv_d = nc.dram_tensor("wv", [8, 128, 256], dt.bfloat16, kind="ExternalInput").ap()
    wpt_d = nc.dram_tensor("wpt", [2 * 128, D], dt.bfloat16, kind="ExternalInput").ap()
    mask_d = nc.dram_tensor("masks", [128, 256], dt.bfloat16, kind="ExternalInput").ap()
    ones_d = nc.dram_tensor("ones", [128, 64], dt.float32r, kind="ExternalInput").ap()
    out_d = nc.dram_tensor("outt", [D, N], dt.bfloat16, kind="ExternalOutput").ap()
    dbg = None
    if os.environ.get("KDBG") == "1":
        dbg = {
            "sa": nc.dram_tensor("dbg_sa", [2, 128, N], dt.bfloat16, kind="ExternalOutput").ap(),
            "kqvT": nc.dram_tensor("dbg_kqvT", [4, 128, N], dt.bfloat16, kind="ExternalOutput").ap(),
            "vp": nc.dram_tensor("dbg_vp", [128, 16 * 264], dt.bfloat16, kind="ExternalOutput").ap(),
        }

    with tile.TileContext(nc) as tc:
        _emit(nc, tc, xf_d, xt_d, w_d, b_d, wv_d, wpt_d, mask_d, ones_d, out_d, dbg)

    nc.compile()
    return nc


def _emit(nc, tc, xf_d, xt_d, w_d, b_d, wv_d, wpt_d, mask_d, ones_d, out_d, dbg=None):
    from contextlib import ExitStack

    dt = mybir.dt
    ctx = ExitStack()
    with ctx:
        consts = ctx.enter_context(tc.tile_pool(name="consts", bufs=1))
        work = ctx.enter_context(tc.tile_pool(name="work", bufs=1))

        # ---- staged constant loads, consumption order ----
        w_sb = [None] * 4

        def load_w(mc):
            w_sb[mc] = consts.tile([128, 4, 2, 128], dt.float8e4, name=f"w{mc}", tag=f"w{mc}")
            nc.sync.dma_start(w_sb[mc][:], w_d[mc].transpose([1, 0, 2, 3]))

        # fp8 x^T folded, for k/q DR matmuls: quarters then half
        xf_q = [[None] * 2 for _ in range(4)]
        xf_half = [None] * 4

        def load_xf_quarter(sub):
            for dc in range(4):
                t = consts.tile([128, 2, NCH], dt.float8e4,
                                name=f"xfq{dc}_{sub}", tag=f"xfq{dc}_{sub}")
                nc.sync.dma_start(t[:], xf_d[dc][:, :, sub * NCH:(sub + 1) * NCH])
                xf_q[dc][sub] = t

        def load_xf_half():
            for dc in range(4):
                t = consts.tile([128, 2, N // 2], dt.float8e4,
                                name=f"xfh{dc}", tag=f"xfh{dc}")
                nc.sync.dma_start(t[:], xf_d[dc][:, :, N // 2:])
                xf_half[dc] = t

        def xf_ap(dc, jj):
            if jj < 2:
                return xf_q[dc][jj][:]
            return xf_half[dc][:, :, (jj - 2) * NCH:(jj - 1) * NCH]

        # bf16 x^T, for v-direct: quarters then half
        xt_q = [[None] * 2 for _ in range(8)]
        xt_sb = [None] * 8

        def load_xt_quarter(sub):
            for dc in range(8):
                t = consts.tile([128, NCH], dt.bfloat16,
                                name=f"xtq{dc}_{sub}", tag=f"xtq{dc}_{sub}")
                nc.sync.dma_start(
                    t[:], xt_d[dc * 128:(dc + 1) * 128,
                               sub * NCH:(sub + 1) * NCH])
                xt_q[dc][sub] = t

        def load_xt_half():
            for dc in range(8):
                t = consts.tile([128, N // 2], dt.bfloat16,
                                name=f"xth{dc}", tag=f"xth{dc}")
                nc.sync.dma_start(
                    t[:], xt_d[dc * 128:(dc + 1) * 128, N // 2:])
                xt_sb[dc] = t

        def xt_ap(dc, jj):
            if jj < 2:
                return xt_q[dc][jj][:]
            return xt_sb[dc][:, (jj - 2) * NCH:(jj - 1) * NCH]

        load_w(0)
        load_w(1)
        load_xf_quarter(0)
        b_sb = consts.tile([128, 4], dt.float32, name="ball", tag="ball")
        nc.sync.dma_start(b_sb[:], b_d.transpose([1, 0]))
        wv_sb = consts.tile([128, 8, 256], dt.bfloat16, name="wv", tag="wv")
        nc.sync.dma_start(wv_sb[:], wv_d.transpose([1, 0, 2]))
        load_xt_quarter(0)
        load_w(2)
        load_w(3)
        mask_sb = consts.tile([128, 256], dt.bfloat16, name="maskall", tag="maskall")
        nc.sync.dma_start(mask_sb[:], mask_d[:])
        ones_sb = consts.tile([128, 64], dt.float32r, name="ones", tag="ones")
        nc.sync.dma_start(ones_sb[:], ones_d[:])
        load_xf_quarter(1)
        load_xt_quarter(1)
        load_xf_half()
        load_xt_half()
        wpt_sb = []
        for kc in range(2):
            t = consts.tile([128, D], dt.bfloat16, name=f"wpt{kc}", tag=f"wpt{kc}")
            nc.sync.dma_start(t[:], wpt_d[kc * 128:(kc + 1) * 128, :])
            wpt_sb.append(t)

        # ---- persistent work tiles ----
        # kqvT[2p+sec]: sec 0=k, 1=q for pair p; [he-dh64 | ho-dh64] rows
        kqvT = [work.tile([128, N], dt.bfloat16, name=f"kqvT{i}", tag=f"kqvT{i}")
                for i in range(4)]
        # vp: [m-part, mb, 4*(64 dh + ones + pad)] bf16
        vp = work.tile([128, NMB, 4 * 66], dt.bfloat16, name="vp", tag="vp")
        saT = [work.tile([128, N], dt.bfloat16, name=f"saT{kc}", tag=f"saT{kc}")
               for kc in range(2)]
        for h in range(G):
            nc.gpsimd.memset(vp[:, :, 66 * h + 64:66 * h + 65], 1.0)

        # ---- pools ----
        ps = ctx.enter_context(tc.tile_pool(name="ps", bufs=2, space="PSUM"))
        pu = ctx.enter_context(tc.tile_pool(name="pu", bufs=2, space="PSUM"))
        pp = ctx.enter_context(tc.tile_pool(name="pp", bufs=2, space="PSUM"))
        pPool = ctx.enter_context(tc.tile_pool(name="pP", bufs=9))
        pun = ctx.enter_context(tc.tile_pool(name="pun", bufs=5))
        paux = ctx.enter_context(tc.tile_pool(name="paux", bufs=6))
        pout = ctx.enter_context(tc.tile_pool(name="pout", bufs=6))

        def emit_kq_group(mc, jj):
            """mc = 2*pair + sec (sec 0=k, 1=q). fp8 DR, bf16 copy-out+bias."""
            ps_t = pp.tile([128, NCH], dt.float32, tag="pp", name="kqp")
            for dc in range(4):
                nc.tensor.matmul(
                    ps_t[:],
                    w_sb[mc][:, dc, :, :],
                    xf_ap(dc, jj),
                    start=(dc == 0), stop=(dc == 3),
                    perf_mode=DR,
                )
            nc.vector.tensor_scalar_add(
                kqvT[mc][:, jj * NCH:(jj + 1) * NCH], ps_t[:], b_sb[:, mc:mc + 1])

        def emit_vd(mb):
            """v-direct: vp[:, mb, 4x64] = x^T-block.T @ Wv."""
            ps_t = pp.tile([128, 256], dt.float32, tag="pp", name="vdp")
            jj = mb // 4
            for dc in range(8):
                nc.tensor.matmul(
                    ps_t[:],
                    xt_ap(dc, jj)[:, (mb % 4) * 128:(mb % 4 + 1) * 128],
                    wv_sb[:, dc, :],
                    start=(dc == 0), stop=(dc == 7),
                )
            nc.vector.tensor_copy(
                vp[:, mb, :].rearrange("p (h e) -> p h e", h=4)[:, :, 0:64],
                ps_t[:].rearrange("p (h e) -> p h e", h=4))

        def head_slices(h):
            p, o = h // 2, (h % 2) * 64
            kT = kqvT[2 * p][o:o + 64, :]
            qT = kqvT[2 * p + 1][o:o + 64, :]
            return kT, qT, o

        def emit_proj_oc(j, oc, scalar_copy=False):
            nsl = slice(j * NCH, (j + 1) * NCH)
            pp_t = pp.tile([128, NCH], dt.float32, tag="pp", name="pp_t")
            for kc in range(2):
                nc.tensor.matmul(
                    pp_t[:],
                    wpt_sb[kc][:, oc * 128:(oc + 1) * 128],
                    saT[kc][:, nsl],
                    start=(kc == 0), stop=(kc == 1),
                )
            o_t = pout.tile([128, NCH], dt.bfloat16, tag="o", name="o_t")
            if scalar_copy:
                nc.scalar.copy(o_t[:], pp_t[:])
            else:
                nc.vector.tensor_copy(o_t[:], pp_t[:])
            nc.sync.dma_start(out_d[oc * 128:(oc + 1) * 128, nsl], o_t[:])

        def norm_steps(h, j, u_t, dve_mul=False):
            kc, row = h // 2, (h % 2) * 64
            nsl = slice(j * NCH, (j + 1) * NCH)
            st = {}
            mul_eng = nc.vector if dve_mul else nc.gpsimd

            def s1():
                st["u_sb"] = pun.tile([65, NCH], dt.float32r, tag="un", name="usb")
                nc.vector.tensor_copy(st["u_sb"][0:65, :], u_t[0:65, :])

            def s2():
                bt = ps.tile([128, 2 * NCH], dt.float32, tag="s2", name="bcp")
                st["bcp"] = bt[0:64, 0:NCH]
                nc.tensor.matmul(
                    st["bcp"],
                    ones_sb[64:65, 0:64],
                    st["u_sb"][64:65, :],
                    start=True, stop=True,
                )

            def s3():
                st["rc"] = paux.tile([64, NCH], dt.float32, tag="rc", name="rc")
                nc.vector.reciprocal_approx_fast(st["rc"][:], st["bcp"])

            def s4():
                u_f32 = st["u_sb"][0:64, :].bitcast(dt.float32)
                if row == 0:
                    mul_eng.tensor_mul(saT[kc][0:64, nsl], u_f32, st["rc"][:])
                else:
                    tmp = paux.tile([64, NCH], dt.bfloat16, tag="tmp", name="tmp")
                    mul_eng.tensor_mul(tmp[:], u_f32, st["rc"][:])
                    nc.sync.dma_start(saT[kc][64:128, nsl], tmp[:])

            return [s1, s2, s3, s4]

        def emit_attn_chunk(j, p, fillers, norm_prev, gated=(), norm_dve=False):
            """Attention chunk j for pair p. `fillers`: dependency-free PE work.
            `norm_prev`: deferred norm steps of the previous chunk. `gated`:
            fillers that must be EMITTED only after all norm_prev steps (they
            read tiles norm_prev writes)."""
            nm = 4 * (j + 1)
            pair = (2 * p, 2 * p + 1)
            u_t = {h: pu.tile([65, NCH], dt.float32, tag="u", name=f"u{h}")
                   for h in pair}
            p_tiles = {h: [None] * nm for h in pair}
            offs = [0] * nm
            from itertools import zip_longest
            units = [u for pair_ in zip_longest(list(norm_prev), list(fillers))
                     for u in pair_ if u is not None]
            units += list(gated)
            total = len(units)
            acc = [0]

            def drain_share():
                acc[0] += total
                while acc[0] >= nm and units:
                    acc[0] -= nm
                    units.pop(0)()

            def pair_view(t, off):
                return t[:, :].rearrange("p (i n) -> p i n", i=2)[:, :, off:]

            def emit_s_pair(mi):
                r = mi - 4 * j
                off = 128 * r if r > 0 else 0
                offs[mi] = off
                s2t = ps.tile([128, 2 * NCH], dt.float32, tag="s2", name="s2t")
                for idx, h in enumerate(pair):
                    kT, qT, _ = head_slices(h)
                    nc.tensor.matmul(
                        s2t[:, idx * NCH + off:(idx + 1) * NCH],
                        kT[:, mi * 128:(mi + 1) * 128],
                        qT[:, j * NCH + off:(j + 1) * NCH],
                        start=True, stop=True, skip_group_check=True,
                    )
                p_t = pPool.tile([128, 2 * NCH], dt.bfloat16, tag="p", name="p_t")
                nc.scalar.activation(pair_view(p_t, off), pair_view(s2t, off),
                                     AF.Exp, scale=0.125 / (ALPHA * ALPHA))
                if r >= 0:
                    # in-place triangular mask on the 128-wide diagonal band
                    band = pair_view(p_t, off)[:, :, 0:128]
                    nc.vector.tensor_mul(
                        band, band,
                        mask_sb[:, :].rearrange("p (i n) -> p i n", i=2))
                for h in pair:
                    p_tiles[h][mi] = p_t

            def emit_pv(h, mi):
                off = offs[mi]
                idx = h % 2
                nc.tensor.matmul(
                    u_t[h][:, off:],
                    vp[:, mi, 66 * h:66 * h + 65],
                    p_tiles[h][mi][:, idx * NCH + off:(idx + 1) * NCH],
                    start=(mi == 0), stop=(mi == nm - 1),
                    skip_group_check=True,
                )

            depth = 4
            for mi in range(nm):
                drain_share()
                emit_s_pair(mi)
                if mi >= depth:
                    for h in pair:
                        emit_pv(h, mi - depth)
            for mi in range(max(nm - depth, 0), nm):
                for h in pair:
                    emit_pv(h, mi)
            while units:
                units.pop(0)()

            return [st for h in pair
                    for st in norm_steps(h, j, u_t[h], dve_mul=norm_dve)]

        # ---- window schedule ----
        KQ = emit_kq_group
        VD = emit_vd

        def F(fn, *a):
            return lambda: fn(*a)

        # A-mini: enough for attn (p0, 0): pair0 k/q at jj=0
        KQ(0, 0); KQ(1, 0)

        windows = [
            (0, 0, [F(VD, 0), F(VD, 1), F(VD, 2), F(VD, 3),
                    F(KQ, 2, 0), F(KQ, 3, 0), F(KQ, 0, 1), F(KQ, 1, 1)], []),
            (0, 1, [F(VD, 4), F(VD, 5), F(VD, 6), F(VD, 7),
                    F(KQ, 2, 1), F(KQ, 3, 1), F(KQ, 0, 2), F(KQ, 1, 2)], []),
            (0, 2, [F(VD, 8), F(VD, 9), F(VD, 10), F(VD, 11),
                    F(KQ, 2, 2), F(KQ, 3, 2), F(KQ, 0, 3), F(KQ, 1, 3)], []),
            (0, 3, [F(VD, 12), F(VD, 13), F(VD, 14), F(VD, 15),
                    F(KQ, 2, 3), F(KQ, 3, 3)], []),
            (1, 1, [], []),
            (1, 2, [], [F(emit_proj_oc, 1, oc) for oc in range(4)]),
            (1, 3, [F(emit_proj_oc, 1, oc) for oc in range(4, 8)],
                   [F(emit_proj_oc, 2, oc) for oc in range(6)]),
            (1, 0, [F(emit_proj_oc, 2, oc) for oc in range(6, 8)],
                   [F(emit_proj_oc, 3, oc, oc % 2 == 1) for oc in range(8)]),
        ]

        pending_norm = []
        for wi, (p, j, fills, gated) in enumerate(windows):
            pending_norm = emit_attn_chunk(j, p, fills, pending_norm, gated,
                                           norm_dve=False)
        # tail: last chunk's norm (e/o interleaved) + its projection
        e_st, o_st = pending_norm[0:4], pending_norm[4:8]
        for a, b in zip(e_st, o_st):
            a(); b()
        for oc in range(8):
            emit_proj_oc(0, oc, scalar_copy=(oc % 2 == 1))

        if dbg is not None:
            for kc in range(2):
                nc.sync.dma_start(dbg["sa"][kc], saT[kc][:])
            for i in range(4):
                nc.sync.dma_start(dbg["kqvT"][i], kqvT[i][:])
            nc.sync.dma_start(dbg["vp"], vp.rearrange("p a b -> p (a b)"))


def _host_prep(x, W_kqv, b_kqv, W_proj):
    """Build the 8 per-core input maps."""
    x = np.asarray(x, dtype=f32)
    W_kqv = np.asarray(W_kqv, dtype=f32)
    b_kqv = np.asarray(b_kqv, dtype=f32)
    W_proj = np.asarray(W_proj, dtype=f32)

    # 128-wide triangular diag mask, duplicated for the head pair
    mm = np.arange(128)[:, None]
    cc = np.arange(128)[None, :]
    m1 = (cc >= mm).astype(bf16)
    masks = np.concatenate([m1, m1], axis=1)  # [128, 256]

    in_maps = []
    for c in range(NCORES):
        b, g = c // 4, c % 4
        heads = [4 * g + i for i in range(4)]
        # k/q groups: mc = 2*pair + sec, cols [he-dh64 | ho-dh64], fp8 fold
        wtiles, btiles = [], []
        for p in range(2):
            he, ho = heads[2 * p], heads[2 * p + 1]
            for sec in range(2):  # k, q
                blk = np.concatenate(
                    [W_kqv[h][:, sec * 64:(sec + 1) * 64] for h in (he, ho)],
                    axis=1) * ALPHA  # [1024, 128]
                wtiles.append(blk.reshape(4, 2, 128, 128).transpose(0, 2, 1, 3)
                              .astype(f8))
                btiles.append(np.concatenate(
                    [b_kqv[h][sec * 64:(sec + 1) * 64] for h in (he, ho)]
                ).astype(f32) * ALPHA)
        # Wv: [8, 128, 256], cols = 4 heads x 64 (UNSCALED, bias dropped)
        wv = np.concatenate([W_kqv[h][:, 128:192] for h in heads], axis=1)
        wv = wv.reshape(8, 128, 256).astype(bf16)
        xt = np.ascontiguousarray(x[b].T)  # [1024, 2048]
        xf = xt.reshape(4, 2, 128, N).transpose(0, 2, 1, 3).astype(f8)
        in_maps.append({
            "xf": xf,
            "xt": xt.astype(bf16),
            "w": np.stack(wtiles),
            "bvec": np.stack(btiles),
            "wv": wv,
            "wpt": np.ascontiguousarray(W_proj[:, 256 * g:256 * (g + 1)].T).astype(bf16),
            "masks": masks,
            "ones": np.ones((128, 64), dtype=f32),
        })
    return in_maps


def run(x, W_kqv, b_kqv, W_proj, b_proj, trace=False, trace_cores=None):
    if "nc" not in _cache:
        _cache["nc"] = _build_program()
    nc = _cache["nc"]
    in_maps = _host_prep(x, W_kqv, b_kqv, W_proj)
    res = bass_utils.run_bass_kernel_spmd(
        nc, in_maps, core_ids=list(range(NCORES)),
        trace=trace, trace_cores=trace_cores,
    )
    b_proj = np.asarray(b_proj, dtype=f32)
    W_proj_f = np.asarray(W_proj, dtype=f32)
    # v bias folded: sum_m attn = 1 -> sa += bv; out += bv_all @ W_proj.T
    bv_all = np.asarray(b_kqv, dtype=f32)[:, 128:192].reshape(-1)  # [1024]
    b_eff = b_proj + bv_all @ W_proj_f.T
    out = np.zeros((B, N, D), dtype=f32)
    for b in range(B):
        acc = res.results[4 * b]["outt"].astype(f32)
        for g in range(1, 4):
            acc = acc + res.results[4 * b + g]["outt"].astype(f32)
        out[b] = acc.T + b_eff[None, :]
    return out, res


def kernel(x, W_kqv, b_kqv, W_proj, b_proj):
    out, _ = run(x, W_kqv, b_kqv, W_proj, b_proj, trace=False)
    return out
